# revision 9
# baseline (speedup 1.0000x reference)
"""Trainium2 Bass kernel for EpsModel.

Math: for each 2x2 batch matrix X (B of them) and a fixed 2x2 A = A0inv:
    T  = X @ A
    ft = A @ (I - T) @ (I + T@T) = A @ (I - T + T^2 - T^3)
Cayley-Hamilton for 2x2: T^2 = s*T - d*I  (s = tr T, d = det T), so
    I - T + T^2 - T^3 = alpha*I + beta*T
    alpha = 1 + d*(s - 1)
    beta  = -1 + s - s^2 + d
and  ft = A @ (alpha*I + beta*T)  with  d = detA * (x0*x3 - x1*x2),
s = t0 + t3.  Pure streaming elementwise pipeline on DVE + ACT in the
natural interleaved layout (quad = [x0,x1,x2,x3] contiguous).

Sharding: data-parallel over the leading batch dim across 8 cores.
"""

import sys

if "/opt/trn_rl_repo" not in sys.path:
    sys.path.insert(0, "/opt/trn_rl_repo")

import numpy as np

B_TOTAL = 8388608
N_CORES = 8
BPC = B_TOTAL // N_CORES        # batch elements per core

Q = 1024                        # quads (batch elems) per partition per chunk
CHUNK = 128 * Q
N_CHUNKS = BPC // CHUNK
FREE = 4 * Q

assert BPC % CHUNK == 0


def build_nc(a, b, c, d, n_chunks=N_CHUNKS, q=Q, reps=1,
             use_pool=False, square_trick=True, bcast_g=True):
    """Build the per-core Bass kernel with A entries folded in as immediates.

    reps>1 wraps the whole pipeline in a For_i hardware loop re-running the
    same work (used only for device-side timing amortization in test.py).
    use_pool: run some tensor_tensor ops on GpSimd to offload the DVE.
    square_trick: s - s^2 = 0.25 - (s-0.5)^2 via ACT Square (saves a DVE op).
    bcast_g: merge the four G-mults into two via a broadcast beta AP.
    """
    import concourse.mybir as mybir
    import concourse.bacc as bacc
    from concourse import tile
    from contextlib import ExitStack

    f32 = mybir.dt.float32
    Alu = mybir.AluOpType
    Act = mybir.ActivationFunctionType

    a = float(a); b = float(b); c = float(c); d = float(d)
    detA = a * d - b * c
    free = 4 * q

    nc = bacc.Bacc("TRN2", target_bir_lowering=False, debug=False)
    x_d = nc.dram_tensor("x", [n_chunks, 128, free], f32, kind="ExternalInput").ap()
    o_d = nc.dram_tensor("out", [n_chunks, 128, free], f32, kind="ExternalOutput").ap()

    def pairs(ap):
        return ap.rearrange("p (q f) -> p q f", f=2)

    def quads(ap):
        return ap.rearrange("p (q f) -> p q f", f=4)

    with tile.TileContext(nc) as tc, ExitStack() as ctx:
        io = ctx.enter_context(tc.tile_pool(name="io", bufs=2))
        pp = ctx.enter_context(tc.tile_pool(name="pp", bufs=2))
        qq = ctx.enter_context(tc.tile_pool(name="qq", bufs=2))

        rep_ctx = tc.For_i(0, reps, 1) if reps > 1 else None
        if rep_ctx is not None:
            rep_ctx.__enter__()

        for ci in range(n_chunks):
            x = io.tile([128, free], f32, tag="x", name=f"x{ci}")
            nc.sync.dma_start(x, x_d[ci])
            xq = quads(x)
            xp = pairs(x)
            xe = xp[:, :, 0]                     # [x0, x2] stream  [128, 2q]
            xo = xp[:, :, 1]                     # [x1, x3] stream
            x0 = xq[:, :, 0]; x1 = xq[:, :, 1]   # [128, q] each
            x2 = xq[:, :, 2]; x3 = xq[:, :, 3]

            # ---- T = X @ A as even/odd pair streams (ACT scale + DVE fused MAC)
            # te = [t0, t2] = a*xe + c*xo ; to = [t1, t3] = b*xe + d*xo
            te = pp.tile([128, 2 * q], f32, tag="te", name=f"te{ci}")
            nc.scalar.activation(te, xo, Act.Copy, bias=0.0, scale=c)
            nc.vector.scalar_tensor_tensor(te, xe, a, te, Alu.mult, Alu.add)
            to = pp.tile([128, 2 * q], f32, tag="to", name=f"to{ci}")
            nc.scalar.activation(to, xo, Act.Copy, bias=0.0, scale=d)
            nc.vector.scalar_tensor_tensor(to, xe, b, to, Alu.mult, Alu.add)

            tep = pairs(te); top = pairs(to)
            t0 = tep[:, :, 0]; t2 = tep[:, :, 1]
            t1 = top[:, :, 0]; t3 = top[:, :, 1]

            # ---- scalar streams ----
            eng2 = nc.gpsimd if use_pool else nc.vector

            s = qq.tile([128, q], f32, tag="s", name=f"s{ci}")
            nc.vector.tensor_tensor(s, t0, t3, Alu.add)

            m1 = qq.tile([128, q], f32, tag="qa", name=f"m1_{ci}")
            eng2.tensor_tensor(m1, x0, x3, Alu.mult)
            m2 = qq.tile([128, q], f32, tag="qb", name=f"m2_{ci}")
            eng2.tensor_tensor(m2, x1, x2, Alu.mult)
            dx = qq.tile([128, q], f32, tag="dx", name=f"dx{ci}")
            eng2.tensor_tensor(dx, m1, m2, Alu.subtract)

            sm1 = qq.tile([128, q], f32, tag="qa", name=f"sm1_{ci}")
            nc.scalar.activation(sm1, s, Act.Copy, bias=-1.0, scale=1.0)

            dsm = qq.tile([128, q], f32, tag="qb", name=f"dsm{ci}")
            eng2.tensor_tensor(dsm, dx, sm1, Alu.mult)
            alpha = qq.tile([128, q], f32, tag="alpha", name=f"al{ci}")
            nc.scalar.activation(alpha, dsm, Act.Copy, bias=1.0, scale=detA)

            if square_trick:
                # s - s^2 = 0.25 - (s-0.5)^2 ; beta = d - (s-0.5)^2 - 0.75
                # Square(1 - 2s) = 4*(s-0.5)^2 (bias 1.0 is a preregistered
                # const AP; -0.5 is not), rescaled by 0.25 in the Copy.
                v2 = qq.tile([128, q], f32, tag="qa", name=f"v2_{ci}")
                nc.scalar.activation(v2, s, Act.Square, bias=1.0, scale=-2.0)
                v3 = qq.tile([128, q], f32, tag="qc", name=f"v3_{ci}")
                nc.scalar.activation(v3, v2, Act.Copy, bias=0.75, scale=0.25)
                beta = qq.tile([128, q], f32, tag="beta", name=f"be{ci}")
                nc.vector.scalar_tensor_tensor(
                    beta, dx, detA, v3, Alu.mult, Alu.subtract
                )
            else:
                v = qq.tile([128, q], f32, tag="qb", name=f"v{ci}")
                nc.vector.tensor_tensor(v, s, sm1, Alu.mult)      # s^2 - s
                r = qq.tile([128, q], f32, tag="qa", name=f"r{ci}")
                nc.vector.scalar_tensor_tensor(
                    r, dx, detA, v, Alu.mult, Alu.subtract
                )
                beta = qq.tile([128, q], f32, tag="beta", name=f"be{ci}")
                nc.scalar.activation(beta, r, Act.Copy, bias=-1.0, scale=1.0)

            # ---- G = alpha*I + beta*T, computed in place over T ----
            if bcast_g:
                bb2 = beta.unsqueeze(2).broadcast_to([128, q, 2])
                nc.vector.tensor_tensor(tep, bb2, tep, Alu.mult)
                nc.vector.tensor_tensor(top, bb2, top, Alu.mult)
            else:
                nc.vector.tensor_tensor(t0, beta, t0, Alu.mult)
                nc.vector.tensor_tensor(t1, beta, t1, Alu.mult)
                nc.vector.tensor_tensor(t2, beta, t2, Alu.mult)
                nc.vector.tensor_tensor(t3, beta, t3, Alu.mult)
            nc.vector.tensor_tensor(t0, t0, alpha, Alu.add)       # g0
            nc.vector.tensor_tensor(t3, t3, alpha, Alu.add)       # g3
            g0, g1, g2, g3 = t0, t1, t2, t3

            # ---- ft = A @ G ----
            # f0 = a*g0 + b*g2 ; f1 = a*g1 + b*g3
            # f2 = c*g0 + d*g2 ; f3 = c*g1 + d*g3
            out = io.tile([128, free], f32, tag="o", name=f"o{ci}")
            oq = quads(out)
            f0 = oq[:, :, 0]; f1 = oq[:, :, 1]
            f2 = oq[:, :, 2]; f3 = oq[:, :, 3]

            if abs(b) >= abs(d):
                # scale g2/g3 by b; f0/f1 fused; f2/f3 use ratio d/b
                rt = d / b if b != 0.0 else 0.0
                nc.scalar.activation(g2, g2, Act.Copy, bias=0.0, scale=b)
                nc.vector.scalar_tensor_tensor(f0, g0, a, g2, Alu.mult, Alu.add)
                nc.scalar.activation(g0, g0, Act.Copy, bias=0.0, scale=c)
                nc.vector.scalar_tensor_tensor(f2, g2, rt, g0, Alu.mult, Alu.add)
                nc.scalar.activation(g3, g3, Act.Copy, bias=0.0, scale=b)
                nc.vector.scalar_tensor_tensor(f1, g1, a, g3, Alu.mult, Alu.add)
                nc.scalar.activation(g1, g1, Act.Copy, bias=0.0, scale=c)
                nc.vector.scalar_tensor_tensor(f3, g3, rt, g1, Alu.mult, Alu.add)
            else:
                # scale g2/g3 by d; f2/f3 fused; f0/f1 use ratio b/d
                rt = b / d
                nc.scalar.activation(g2, g2, Act.Copy, bias=0.0, scale=d)
                nc.vector.scalar_tensor_tensor(f2, g0, c, g2, Alu.mult, Alu.add)
                nc.scalar.activation(g0, g0, Act.Copy, bias=0.0, scale=a)
                nc.vector.scalar_tensor_tensor(f0, g2, rt, g0, Alu.mult, Alu.add)
                nc.scalar.activation(g3, g3, Act.Copy, bias=0.0, scale=d)
                nc.vector.scalar_tensor_tensor(f3, g1, c, g3, Alu.mult, Alu.add)
                nc.scalar.activation(g1, g1, Act.Copy, bias=0.0, scale=a)
                nc.vector.scalar_tensor_tensor(f1, g3, rt, g1, Alu.mult, Alu.add)

            nc.sync.dma_start(o_d[ci], out)

        if rep_ctx is not None:
            rep_ctx.__exit__(None, None, None)

    nc.compile()
    return nc


def kernel(x, A0inv):
    x = np.ascontiguousarray(np.asarray(x, dtype=np.float32))
    A = np.asarray(A0inv, dtype=np.float32)
    a, b = float(A[0, 0]), float(A[0, 1])
    c, d = float(A[1, 0]), float(A[1, 1])

    from concourse.bass_utils import run_bass_kernel_spmd

    nc = build_nc(a, b, c, d)

    shards = x.reshape(N_CORES, N_CHUNKS, 128, FREE)
    in_maps = [{"x": shards[i]} for i in range(N_CORES)]
    res = run_bass_kernel_spmd(nc, in_maps, list(range(N_CORES)))
    out = np.concatenate(
        [r["out"].reshape(BPC, 2, 2) for r in res.results], axis=0
    )
    return out.astype(np.float32, copy=False)


# revision 15
# speedup vs baseline: 1.0594x; 1.0594x over previous
"""Trainium2 Bass kernel for EpsModel.

Math: for each 2x2 batch matrix X (B of them) and a fixed 2x2 A = A0inv:
    T  = X @ A
    ft = A @ (I - T) @ (I + T@T) = A @ (I - T + T^2 - T^3)
Cayley-Hamilton for 2x2: T^2 = s*T - d*I  (s = tr T, d = det T), so
    I - T + T^2 - T^3 = alpha*I + beta*T
    alpha = 1 + d*(s - 1)
    beta  = -1 + s - s^2 + d      (s - s^2 = 0.25 - (s-0.5)^2 via ACT Square)
and  ft = A @ G,  G = alpha*I + beta*T,  d = detA * (x0*x3 - x1*x2).

Everything is elementwise over the batch; the kernel is a streaming
pipeline over [128, 4q] fp32 tiles in the natural interleaved layout.
T is stored quad-wise as [t0, t3, t1, t2] so that:
  - the four beta*T multiplies are ONE broadcast tensor_tensor,
  - both diagonal "+alpha" adds are ONE broadcast tensor_tensor,
  - s = t0 + t3 reads adjacent elements.
Work is split DVE (2-src fused ops) / ACT (1-src affine) / optionally PE
(scaled-identity accumulating matmuls for half the final combine and s).

Sharding: data-parallel over the leading batch dim across 8 cores.
"""

import sys

if "/opt/trn_rl_repo" not in sys.path:
    sys.path.insert(0, "/opt/trn_rl_repo")

import numpy as np

B_TOTAL = 8388608
N_CORES = 8
BPC = B_TOTAL // N_CORES        # batch elements per core

Q = 1024                        # quads (batch elems) per partition per chunk
CHUNK = 128 * Q
N_CHUNKS = BPC // CHUNK
FREE = 4 * Q

assert BPC % CHUNK == 0

PE_OFF = 1                      # 0: none, 1: +ft(f0,f2) on PE, 2: +s on PE


def build_nc(a, b, c, d, n_chunks=N_CHUNKS, q=Q, reps=1, pe_off=PE_OFF):
    """Build the per-core Bass kernel with A entries folded in as immediates.

    reps>1 wraps the pipeline in a For_i hardware loop re-running the same
    work (used only for device-side timing amortization in test.py).
    """
    import concourse.mybir as mybir
    import concourse.bacc as bacc
    from concourse import tile, masks
    from contextlib import ExitStack

    f32 = mybir.dt.float32
    Alu = mybir.AluOpType
    Act = mybir.ActivationFunctionType

    a = float(a); b = float(b); c = float(c); d = float(d)
    detA = a * d - b * c
    free = 4 * q
    PW = 512                    # one PSUM bank of fp32

    nc = bacc.Bacc("TRN2", target_bir_lowering=False, debug=False)
    x_d = nc.dram_tensor("x", [n_chunks, 128, free], f32, kind="ExternalInput").ap()
    o_d = nc.dram_tensor("out", [n_chunks, 128, free], f32, kind="ExternalOutput").ap()

    def quads(ap):
        return ap.rearrange("p (q f) -> p q f", f=4)

    with tile.TileContext(nc) as tc, ExitStack() as ctx:
        io = ctx.enter_context(tc.tile_pool(name="io", bufs=4))
        pp = ctx.enter_context(tc.tile_pool(name="pp", bufs=2))
        qq = ctx.enter_context(tc.tile_pool(name="qq", bufs=2))

        if pe_off >= 1:
            wts = ctx.enter_context(tc.tile_pool(name="wts", bufs=1))
            fps = ctx.enter_context(tc.tile_pool(name="fps", bufs=4, space="PSUM"))
            ident = wts.tile([128, 128], f32, tag="ident", name="ident")
            masks.make_identity(nc, ident)
            wI = {}
            for nm, val in (("a", a), ("b", b), ("c", c), ("d", d)):
                w = wts.tile([128, 128], f32, tag=f"w{nm}", name=f"w{nm}")
                nc.scalar.activation(w, ident, Act.Copy, bias=0.0, scale=val)
                wI[nm] = w
        if pe_off >= 2:
            sps = ctx.enter_context(tc.tile_pool(name="sps", bufs=2, space="PSUM"))

        rep_ctx = tc.For_i(0, reps, 1) if reps > 1 else None
        if rep_ctx is not None:
            rep_ctx.__enter__()

        for ci in range(n_chunks):
            x = io.tile([128, free], f32, tag="x", name=f"x{ci}")
            nc.sync.dma_start(x, x_d[ci])
            xq = quads(x)
            xe = xq[:, :, 0:4:2]                 # [x0, x2]  [128, q, 2]
            xo = xq[:, :, 1:4:2]                 # [x1, x3]

            # ---- T = X @ A, stored quad-wise as [t0, t3, t1, t2] ----
            # (t0, t2) = a*(x0, x2) + c*(x1, x3) -> positions (0, 3)
            # (t1, t3) = b*(x0, x2) + d*(x1, x3) -> positions (2, 1)
            tt = pp.tile([128, free], f32, tag="tt", name=f"tt{ci}")
            tq = quads(tt)
            te_v = tq[:, :, 0:4:3]               # positions {0, 3}
            to_v = tq[:, :, 2:0:-1]              # positions {2, 1}
            nc.scalar.activation(te_v, xo, Act.Copy, bias=0.0, scale=c)
            nc.vector.scalar_tensor_tensor(te_v, xe, a, te_v, Alu.mult, Alu.add)
            nc.scalar.activation(to_v, xo, Act.Copy, bias=0.0, scale=d)
            nc.vector.scalar_tensor_tensor(to_v, xe, b, to_v, Alu.mult, Alu.add)

            t0 = tq[:, :, 0]; t3 = tq[:, :, 1]
            t1 = tq[:, :, 2]; t2 = tq[:, :, 3]

            # ---- scalar streams ----
            if pe_off >= 2:
                s = sps.tile([128, q], f32, tag="s", name=f"s{ci}")
                for pi in range(q // PW):
                    sl = slice(pi * PW, (pi + 1) * PW)
                    nc.tensor.matmul(s[:, sl], ident, t0[:, sl],
                                     start=True, stop=False)
                    nc.tensor.matmul(s[:, sl], ident, t3[:, sl],
                                     start=False, stop=True)
            else:
                s = qq.tile([128, q], f32, tag="sd", name=f"s{ci}")
                nc.vector.tensor_tensor(s, t0, t3, Alu.add)

            # dx = x0*x3 - x1*x2 via one paired multiply + one subtract
            prod = pp.tile([128, 2 * q], f32, tag="prod", name=f"pr{ci}")
            pv = prod.rearrange("p (q f) -> p q f", f=2)
            nc.vector.tensor_tensor(pv, xq[:, :, 0:2], xq[:, :, 3:1:-1], Alu.mult)
            dx = qq.tile([128, q], f32, tag="sd", name=f"dx{ci}")
            nc.vector.tensor_tensor(dx, pv[:, :, 0], pv[:, :, 1], Alu.subtract)

            sm1 = qq.tile([128, q], f32, tag="sc", bufs=3, name=f"sm1_{ci}")
            nc.scalar.activation(sm1, s, Act.Copy, bias=-1.0, scale=1.0)
            dsm = qq.tile([128, q], f32, tag="sc", bufs=3, name=f"dsm{ci}")
            nc.vector.tensor_tensor(dsm, dx, sm1, Alu.mult)
            alpha = qq.tile([128, q], f32, tag="ab", name=f"al{ci}")
            nc.scalar.activation(alpha, dsm, Act.Copy, bias=1.0, scale=detA)

            # beta = d + s - s^2 - 1 = detA*dx - ((s-0.5)^2 + 0.75)
            v2 = qq.tile([128, q], f32, tag="sc", bufs=3, name=f"v2_{ci}")
            nc.scalar.activation(v2, s, Act.Square, bias=1.0, scale=-2.0)
            v3 = qq.tile([128, q], f32, tag="sc", bufs=3, name=f"v3_{ci}")
            nc.scalar.activation(v3, v2, Act.Copy, bias=0.75, scale=0.25)
            beta = qq.tile([128, q], f32, tag="ab", name=f"be{ci}")
            nc.vector.scalar_tensor_tensor(beta, dx, detA, v3, Alu.mult,
                                           Alu.subtract)

            # ---- G = alpha*I + beta*T in place over tt ----
            b4 = beta.unsqueeze(2).broadcast_to([128, q, 4])
            nc.vector.tensor_tensor(tq, b4, tq, Alu.mult)
            a2 = alpha.unsqueeze(2).broadcast_to([128, q, 2])
            dg = tq[:, :, 0:2]                   # [g0, g3]
            nc.vector.tensor_tensor(dg, a2, dg, Alu.add)

            g0 = tq[:, :, 0]; g3 = tq[:, :, 1]
            g1 = tq[:, :, 2]; g2 = tq[:, :, 3]

            # ---- ft = A @ G, written back into the x tile ----
            # f0 = a*g0 + b*g2 ; f1 = a*g1 + b*g3
            # f2 = c*g0 + d*g2 ; f3 = c*g1 + d*g3
            f0 = xq[:, :, 0]; f1 = xq[:, :, 1]
            f2 = xq[:, :, 2]; f3 = xq[:, :, 3]

            def ft_pair_dve(gA, gB, fT, fB):
                # fT = a*gA + b*gB ; fB = c*gA + d*gB (in-place prescales)
                if max(abs(b), abs(d)) < 1e-30:
                    nc.scalar.activation(fT, gA, Act.Copy, bias=0.0, scale=a)
                    nc.scalar.activation(fB, gA, Act.Copy, bias=0.0, scale=c)
                elif abs(b) >= abs(d):
                    nc.scalar.activation(gB, gB, Act.Copy, bias=0.0, scale=b)
                    nc.vector.scalar_tensor_tensor(fT, gA, a, gB,
                                                   Alu.mult, Alu.add)
                    nc.scalar.activation(gA, gA, Act.Copy, bias=0.0, scale=c)
                    nc.vector.scalar_tensor_tensor(fB, gB, d / b, gA,
                                                   Alu.mult, Alu.add)
                else:
                    nc.scalar.activation(gB, gB, Act.Copy, bias=0.0, scale=d)
                    nc.vector.scalar_tensor_tensor(fB, gA, c, gB,
                                                   Alu.mult, Alu.add)
                    nc.scalar.activation(gA, gA, Act.Copy, bias=0.0, scale=a)
                    nc.vector.scalar_tensor_tensor(fT, gB, b / d, gA,
                                                   Alu.mult, Alu.add)

            if pe_off >= 1:
                # (f0, f2) on the PE; (f1, f3) on DVE/ACT
                for pi in range(q // PW):
                    sl = slice(pi * PW, (pi + 1) * PW)
                    acc0 = fps.tile([128, PW], f32, tag="acc",
                                    name=f"acc0_{ci}_{pi}")
                    nc.tensor.matmul(acc0, wI["a"], g0[:, sl],
                                     start=True, stop=False)
                    nc.tensor.matmul(acc0, wI["b"], g2[:, sl],
                                     start=False, stop=True)
                    nc.scalar.activation(f0[:, sl], acc0, Act.Copy)
                    acc2 = fps.tile([128, PW], f32, tag="acc",
                                    name=f"acc2_{ci}_{pi}")
                    nc.tensor.matmul(acc2, wI["c"], g0[:, sl],
                                     start=True, stop=False)
                    nc.tensor.matmul(acc2, wI["d"], g2[:, sl],
                                     start=False, stop=True)
                    nc.scalar.activation(f2[:, sl], acc2, Act.Copy)
                ft_pair_dve(g1, g3, f1, f3)
            else:
                ft_pair_dve(g0, g2, f0, f2)
                ft_pair_dve(g1, g3, f1, f3)

            nc.sync.dma_start(o_d[ci], x)

        if rep_ctx is not None:
            rep_ctx.__exit__(None, None, None)

    nc.compile()
    return nc


def kernel(x, A0inv):
    x = np.ascontiguousarray(np.asarray(x, dtype=np.float32))
    A = np.asarray(A0inv, dtype=np.float32)
    a, b = float(A[0, 0]), float(A[0, 1])
    c, d = float(A[1, 0]), float(A[1, 1])

    from concourse.bass_utils import run_bass_kernel_spmd

    nc = build_nc(a, b, c, d)

    shards = x.reshape(N_CORES, N_CHUNKS, 128, FREE)
    in_maps = [{"x": shards[i]} for i in range(N_CORES)]
    res = run_bass_kernel_spmd(nc, in_maps, list(range(N_CORES)))
    out = np.concatenate(
        [r["out"].reshape(BPC, 2, 2) for r in res.results], axis=0
    )
    return out.astype(np.float32, copy=False)
